# revision 8
# baseline (speedup 1.0000x reference)
"""Cost-volume kernel for TRN2 (one NeuronCore per batch element).

out[ch, h, w] = (1/81) * sum_c x1[c,h,w] * x2[c, h+i, w+j],
ch = (-9i - j) mod 81, i,j in [-4,4].

Host pre-processing (per core): x1s = (x1/81) as bf16 [C,H,W];
x2p = zero-padded x2 as bf16 [C, H+8, W+8]; ident = eye(128) bf16.

Device pipeline per band of BAND=8 rows (16 bands):
  1. Gram matmuls: per (row-quad wq in 2, w-chunk wch in 8): stationary =
     x1 cols p=32*rho+q <-> pixel (h0+4wq+rho, 32wch+q); moving rhs
     N=480 columns n=12j+r = x2p[row h0+4wq+r, col 32wch+j], j in 40,
     r in 12, K=192 (2 k-split matmuls accumulating in PSUM fp32).
     psum[p, 12(q+b) + rho + 12b'... ] holds channel (a,b) of pixel p at
     n = (12q+rho) + (12b+a).
  2. Evac psum -> G bf16 REVERSED: G[p, 480t + x] = psum_t[p, 479-x],
     so channel value sits at G[p, 480t + d(p) + e_rev], with
     per-partition shift d(p) = 375-12q-rho and e_rev = 12(8-b)+(8-a).
  3. Bounce through DRAM to apply the per-partition bulk shift with big
     descriptors only: dense write G -> scr[band]; 4 shifted reads
     (one per rho, stride 7680-12 on the flat DRAM side) -> D2 with
     D2[p, y] = G[p, d(p) + y].
  4. Per t: PE-transpose D2[:, 480t:480t+105] -> psum [105, 128]
     (partition m = e_rev = 12b'+a'', b'=8-b, a''=8-a), evac to staging
     S fp32 at pixel positions.
  5. Out-DMA: 10 affine rects; rect for fixed a'': channels
     ch = (9a'' + b' - 40) mod 81 ascending with b', src partitions
     12b'+a'' stride 12. Big 8KB runs.
"""

import numpy as np
from contextlib import ExitStack

import concourse.bacc as bacc
import concourse.bass as bass
import concourse.tile as tile
from concourse import mybir

F32 = mybir.dt.float32
BF16 = mybir.dt.bfloat16

C, H, W = 192, 128, 256
NCH = 81
BAND = 8                 # h rows per band
XROWS = BAND + 8         # x2p rows resident per band
XW = W + 8               # padded x2 row length (264)
NQ = BAND // 4           # row-quads per band (2)
NWCH = W // 32           # w-chunks (8)
NT = NQ * NWCH           # gram iters per band (16)
NFREE = 480              # psum free size per iter (40j x 12r)
RUN = 105                # per-pixel channel run (12*8 + 8 + 1)
GF = NT * NFREE          # G free elems per partition per band (7680)
D2F = (NT - 1) * NFREE + RUN  # D2 free elems (7305)
SF = BAND * W            # staging free elems (2048)

# out-DMA rects: (b2, a_lo, na, ch0): src staging partitions
# [12*b2 + a_lo, +na) (contiguous), dst channels ch = (9*a2 + b2 - 40) % 81
# = ch0 + 9*(a2 - a_lo) (stride 9, no wrap within a piece).
RECTS = []
for _b2 in range(9):
    _a0 = -(-(40 - _b2) // 9)  # first a2 with 9*a2 + b2 >= 40
    if _a0 > 0:
        RECTS.append((_b2, 0, _a0, 9 * 0 + _b2 - 40 + 81))
    if _a0 < 9:
        RECTS.append((_b2, _a0, 9 - _a0, 9 * _a0 + _b2 - 40))


def build_cv_kernel(H_=H, reps=1):
    nbands = H_ // BAND
    nc = bacc.Bacc("TRN2", target_bir_lowering=False, debug=False)
    # x1r is host-permuted: x1r[c, band, wq, wch, rho, q] (flattened to
    # [C, H*W]) so each gram iter's 128 stationary columns are contiguous.
    x1 = nc.dram_tensor("x1r", [C, H_ * W], BF16, kind="ExternalInput").ap()
    x2 = nc.dram_tensor("x2p", [C, H_ + 8, XW], BF16, kind="ExternalInput").ap()
    ident = nc.dram_tensor("ident", [128, 128], BF16, kind="ExternalInput").ap()
    out = nc.dram_tensor("out", [NCH, H_, W], F32, kind="ExternalOutput").ap()
    scr = nc.dram_tensor("scr", [nbands, 128, GF], BF16).ap()

    with tile.TileContext(nc) as tc, ExitStack() as ctx:
        const_pool = ctx.enter_context(tc.tile_pool(name="const", bufs=1))
        x_pool = ctx.enter_context(tc.tile_pool(name="x", bufs=2))
        g_pool = ctx.enter_context(tc.tile_pool(name="g", bufs=2))
        d_pool = ctx.enter_context(tc.tile_pool(name="d", bufs=2))
        s_pool = ctx.enter_context(tc.tile_pool(name="s", bufs=2))
        ps_gram = ctx.enter_context(tc.tile_pool(name="psg", bufs=3, space="PSUM"))
        ps_tr = ctx.enter_context(tc.tile_pool(name="pst", bufs=4, space="PSUM"))

        idt = const_pool.tile([128, 128], BF16)
        nc.sync.dma_start(idt[:], ident[:])

        def compute_band(band):
            h0 = band * BAND
            x1lo = x_pool.tile([128, BAND * W], BF16, tag="x1lo")
            x1hi = x_pool.tile([64, BAND * W], BF16, tag="x1hi")
            nc.sync.dma_start(
                x1lo[:], x1[0:128, h0 * W : (h0 + BAND) * W]
            )
            nc.sync.dma_start(
                x1hi[:], x1[128:192, h0 * W : (h0 + BAND) * W]
            )
            x2lo = x_pool.tile([128, XROWS * XW], BF16, tag="x2lo")
            x2hi = x_pool.tile([64, XROWS * XW], BF16, tag="x2hi")
            nc.sync.dma_start(
                x2lo[:].rearrange("p (r w) -> p r w", r=XROWS),
                x2[0:128, h0 : h0 + XROWS, :],
            )
            nc.sync.dma_start(
                x2hi[:].rearrange("p (r w) -> p r w", r=XROWS),
                x2[128:192, h0 : h0 + XROWS, :],
            )

            G = g_pool.tile([128, GF], BF16, tag="G")
            gp = G[:].ap[0][0]
            for t in range(NT):
                wq, wch = divmod(t, NWCH)
                gram = ps_gram.tile([128, NFREE], F32, tag="gram")
                pp = gram[:].ap[0][0]
                for k, (x1t, x2t) in enumerate(((x1lo, x2lo), (x1hi, x2hi))):
                    ncc = x1t[:].ap[0][1]
                    lhsT = bass.AP(
                        tensor=x1t[:].tensor,
                        offset=t * 128,
                        ap=[[BAND * W, ncc], [1, 128]],
                    )
                    rhs = bass.AP(
                        tensor=x2t[:].tensor,
                        offset=(4 * wq) * XW + 32 * wch,
                        ap=[[XROWS * XW, ncc], [1, 40], [XW, 12]],
                    )
                    nc.tensor.matmul(
                        gram[:], lhsT, rhs, start=(k == 0), stop=(k == 1)
                    )
                # reversed evac: G[p, 480t + x] = gram[p, 479 - x]
                dst = bass.AP(
                    tensor=G[:].tensor, offset=t * NFREE, ap=[[gp, 128], [1, NFREE]]
                )
                src = bass.AP(
                    tensor=gram[:].tensor, offset=NFREE - 1,
                    ap=[[pp, 128], [-1, NFREE]],
                )
                if t % 2 == 0:
                    nc.vector.tensor_copy(dst, src)
                else:
                    nc.scalar.copy(dst, src)
            # bounce: dense write, shifted reads (per rho)
            nc.sync.dma_start(scr[band], G[:])
            D2 = d_pool.tile([128, D2F], BF16, tag="D2")
            scr_base = band * 128 * GF
            for rho in range(4):
                src = bass.AP(
                    tensor=scr.tensor,
                    offset=scr_base + 32 * rho * GF + (375 - rho),
                    ap=[[GF - 12, 32], [1, D2F]],
                )
                dst = bass.AP(
                    tensor=D2[:].tensor,
                    offset=32 * rho * D2F,
                    ap=[[D2F, 32], [1, D2F]],
                )
                nc.scalar.dma_start(dst, src)
            return D2

        def output_band(band, D2):
            h0 = band * BAND
            S = s_pool.tile([105, SF], F32, tag="S")
            sp = S[:].ap[0][0]
            d2p = D2[:].ap[0][0]
            for t in range(NT):
                wq, wch = divmod(t, NWCH)
                ptr = ps_tr.tile([105, 128], BF16, tag="ptr")
                tin = bass.AP(
                    tensor=D2[:].tensor, offset=t * NFREE,
                    ap=[[d2p, 128], [1, RUN]],
                )
                nc.tensor.transpose(ptr[:], tin, idt[:])
                dst = bass.AP(
                    tensor=S[:].tensor,
                    offset=(4 * wq) * W + 32 * wch,
                    ap=[[sp, 105], [W, 4], [1, 32]],
                )
                if t % 2 == 0:
                    nc.scalar.copy(dst, ptr[:])
                else:
                    nc.vector.tensor_copy(dst, ptr[:])
            for b2, a_lo, na, ch0 in RECTS:
                src = bass.AP(
                    tensor=S[:].tensor,
                    offset=(12 * b2 + a_lo) * sp,
                    ap=[[sp, na], [1, SF]],
                )
                dst = bass.AP(
                    tensor=out.tensor,
                    offset=ch0 * H_ * W + h0 * W,
                    ap=[[9 * H_ * W, na], [1, SF]],
                )
                nc.sync.dma_start(dst, src)

        for rep in range(reps):
            prev = None
            for band in range(nbands):
                D2 = compute_band(band)
                if prev is not None:
                    output_band(band - 1, prev)
                prev = D2
            output_band(nbands - 1, prev)
    nc.compile()
    return nc


def ref_one(x1, x2):
    """numpy reference for one batch element: x1, x2 [C, H, W] fp32."""
    C_, H_, W_ = x1.shape
    x2p = np.pad(x2, ((0, 0), (4, 4), (4, 4)))
    out = np.zeros((NCH, H_, W_), np.float32)
    for a in range(9):
        for b in range(9):
            ch = (40 - 9 * a - b) % NCH
            out[ch] = (x1 * x2p[:, a : a + H_, b : b + W_]).sum(0) / NCH
    return out


def prepare_in_maps(x1, x2):
    """Host-side prep: returns per-core input dicts for the bass kernel."""
    import ml_dtypes

    bf16 = ml_dtypes.bfloat16
    x1 = np.asarray(x1, dtype=np.float32)
    x2 = np.asarray(x2, dtype=np.float32)
    B, _, H_, W_ = x1.shape
    x1s = (x1 * (1.0 / NCH)).astype(bf16)
    # permute to [c, band, wq, wch, rho, q] and flatten to [C, H*W]
    x1r = (
        x1s.reshape(B, C, H_ // BAND, NQ, 4, NWCH, 32)
        .transpose(0, 1, 2, 3, 5, 4, 6)
        .reshape(B, C, H_ * W_)
    )
    x2p = np.zeros((B, C, H_ + 8, W_ + 8), dtype=bf16)
    x2p[:, :, 4:-4, 4:-4] = x2.astype(bf16)
    eye = np.eye(128, dtype=bf16)
    return [
        {"x1r": np.ascontiguousarray(x1r[i]),
         "x2p": np.ascontiguousarray(x2p[i]),
         "ident": eye}
        for i in range(B)
    ]


_NC_CACHE = {}


def _get_nc():
    if "nc" not in _NC_CACHE:
        _NC_CACHE["nc"] = build_cv_kernel(H_=H)
    return _NC_CACHE["nc"]


def kernel(x1, x2):
    """Full-input entry point: x1, x2 [8, 192, 128, 256] float32 ->
    [8, 81, 128, 256] float32. Data-parallel over batch: core i computes
    batch element i."""
    from concourse import bass_utils

    B = np.asarray(x1).shape[0]
    nc = _get_nc()
    in_maps = prepare_in_maps(x1, x2)
    res = bass_utils.run_bass_kernel_spmd(nc, in_maps, core_ids=list(range(B)))
    return np.stack([res.results[i]["out"] for i in range(B)], axis=0)


# revision 15
# speedup vs baseline: 1.0973x; 1.0973x over previous
"""Cost-volume kernel for TRN2 (one NeuronCore per batch element).

out[ch, h, w] = (1/81) * sum_c x1[c,h,w] * x2[c, h+i, w+j],
ch = (-9i - j) mod 81, i,j in [-4,4].

Host pre-processing (per core): x1r = (x1/81) as bf16, permuted to
[c, band, wq, wch, rho, q] so each gram iter's 128 stationary columns
are contiguous; x2p = zero-padded x2 as bf16 [C, H+8, W+8];
ident = eye(128) bf16.

Device pipeline per band of BAND=8 rows (16 bands):
  1. Gram matmuls: per iter t=(wq, wch): stationary = 128 x1 pixels
     (4 rows x 32 cols), moving N=480 = x2p[row 4wq+r, col 32wch+j],
     j in 40, r in 12; K=192 via 2 accumulating matmuls. Two iters
     share one 2-bank PSUM tile (offsets 0 / 512).
  2. Evac pairs of psum tiles -> G bf16 REVERSED: G[p, 480t + x] =
     psum_t[p, 479-x]; channel (a,b) of pixel p=32rho+q sits at
     G[p, 480t + d(p) + e_rev], d(p) = 375-12q-rho, e_rev=12(8-b)+(8-a).
  3. Bounce via DRAM to apply the per-partition bulk shift with only
     big descriptors: dense write G -> scr[band]; ONE 3-dim shifted
     read -> D2 with D2[p, y] = G[p, d(p) + y].
  4. Per t: PE-transpose D2[:, 480t:+105] -> psum [105, 128] at 128-col
     slices of a shared 1-bank psum tile; ONE batched evac per 8
     transposes into bf16 staging S (4-band window).
  5. Every 4 bands: 18 SWDGE rect DMAs (bf16->fp32 cast) write
     out[ch, h, w]; rect for fixed b': channels ch = (9a''+b'-40)%81
     ascending with stride 9, src partitions 12b'+a_lo.. contiguous.
"""

import numpy as np
from contextlib import ExitStack

import concourse.bacc as bacc
import concourse.bass as bass
import concourse.tile as tile
from concourse import mybir

F32 = mybir.dt.float32
BF16 = mybir.dt.bfloat16

C, H, W = 192, 128, 256
NCH = 81
BAND = 8                 # h rows per band
LBAND = 2 * BAND         # rows per load group (2 bands)
XROWS = LBAND + 8        # x2p rows resident per load group (24)
XW = W + 8               # padded x2 row length (264)
NQ = BAND // 4           # row-quads per band (2)
NWCH = W // 32           # w-chunks (8)
NT = NQ * NWCH           # gram iters per band (16)
NFREE = 480              # psum free size per iter (40j x 12r)
RUN = 105                # per-pixel channel run (12*8 + 8 + 1)
GF = NT * NFREE          # G free elems per partition per band (7680)
D2RD = (NT - 1) * NFREE + RUN  # bounce-read elems per partition (7305)
D2F = (NT - 1) * NFREE + 128  # D2 free elems, padded for 128-wide xbar reads
SCRF = GF                     # scr row pitch
SBANDS = 4               # bands per staging window
SF = BAND * W            # staging elems per band (2048)

# out-DMA rects: (b2, a_lo, na, ch0): src staging partitions
# [12*b2 + a_lo, +na) (contiguous), dst channels ch = (9*a2 + b2 - 40) % 81
# = ch0 + 9*(a2 - a_lo) (stride 9, no wrap within a piece).
RECTS = []
for _b2 in range(9):
    _a0 = -(-(40 - _b2) // 9)  # first a2 with 9*a2 + b2 >= 40
    if _a0 > 0:
        RECTS.append((_b2, 0, _a0, _b2 - 40 + 81))
    if _a0 < 9:
        RECTS.append((_b2, _a0, 9 - _a0, 9 * _a0 + _b2 - 40))


def build_cv_kernel(H_=H, reps=1, tr_mode="pe"):
    nbands = H_ // BAND
    nc = bacc.Bacc("TRN2", target_bir_lowering=False, debug=False)
    x1 = nc.dram_tensor("x1r", [C, H_ * W], BF16, kind="ExternalInput").ap()
    x2 = nc.dram_tensor("x2p", [C, H_ + 8, XW], BF16, kind="ExternalInput").ap()
    ident = nc.dram_tensor("ident", [128, 128], BF16, kind="ExternalInput").ap()
    out = nc.dram_tensor("out", [NCH, H_, W], F32, kind="ExternalOutput").ap()
    scr = nc.dram_tensor("scr", [nbands, 128, SCRF], BF16).ap()

    with tile.TileContext(nc) as tc, ExitStack() as ctx:
        const_pool = ctx.enter_context(tc.tile_pool(name="const", bufs=1))
        x_pool = ctx.enter_context(tc.tile_pool(name="x", bufs=2))
        g_pool = ctx.enter_context(tc.tile_pool(name="g", bufs=2))
        d_pool = ctx.enter_context(tc.tile_pool(name="d", bufs=2))
        s_pool = ctx.enter_context(tc.tile_pool(name="s", bufs=2))
        t_pool = ctx.enter_context(tc.tile_pool(name="t", bufs=2))
        ps_gram = ctx.enter_context(tc.tile_pool(name="psg", bufs=2, space="PSUM"))
        ps_tr = ctx.enter_context(tc.tile_pool(name="pst", bufs=2, space="PSUM"))

        idt = const_pool.tile([128, 128], BF16)
        nc.sync.dma_start(idt[:], ident[:])

        state = {}

        def compute_band(band):
            h0 = band * BAND
            if band % 2 == 0:
                x1t2 = x_pool.tile([128, LBAND * W], BF16, tag="x1lo")
                x1h2 = x_pool.tile([64, LBAND * W], BF16, tag="x1hi")
                nc.sync.dma_start(x1t2[:], x1[0:128, h0 * W : (h0 + LBAND) * W])
                nc.sync.dma_start(x1h2[:], x1[128:192, h0 * W : (h0 + LBAND) * W])
                x2t2 = x_pool.tile([128, XROWS * XW], BF16, tag="x2lo")
                x2h2 = x_pool.tile([64, XROWS * XW], BF16, tag="x2hi")
                nc.sync.dma_start(
                    x2t2[:].rearrange("p (r w) -> p r w", r=XROWS),
                    x2[0:128, h0 : h0 + XROWS, :],
                )
                nc.sync.dma_start(
                    x2h2[:].rearrange("p (r w) -> p r w", r=XROWS),
                    x2[128:192, h0 : h0 + XROWS, :],
                )
                state["x"] = (x1t2, x1h2, x2t2, x2h2)
            x1lo, x1hi, x2lo, x2hi = state["x"]
            bl = band % 2  # band-local index within the load group

            G = g_pool.tile([128, GF], BF16, tag="G")
            gp = G[:].ap[0][0]
            for t0 in range(0, NT, 2):
                gram = ps_gram.tile([128, 1024], F32, tag="gram")
                pp = gram[:].ap[0][0]
                for dt in range(2):
                    t = t0 + dt
                    wq, wch = divmod(t, NWCH)
                    for k, (x1t, x2t) in enumerate(((x1lo, x2lo), (x1hi, x2hi))):
                        ncc = x1t[:].ap[0][1]
                        lhsT = bass.AP(
                            tensor=x1t[:].tensor,
                            offset=(bl * NT + t) * 128,
                            ap=[[LBAND * W, ncc], [1, 128]],
                        )
                        rhs = bass.AP(
                            tensor=x2t[:].tensor,
                            offset=(bl * BAND + 4 * wq) * XW + 32 * wch,
                            ap=[[XROWS * XW, ncc], [1, 40], [XW, 12]],
                        )
                        nc.tensor.matmul(
                            bass.AP(
                                tensor=gram[:].tensor,
                                offset=512 * dt,
                                ap=[[pp, 128], [1, NFREE]],
                            ),
                            lhsT, rhs, start=(k == 0), stop=(k == 1),
                        )
                # one reversed evac for both psum chunks:
                # G[p, 480*(t0+c) + x] = gram[p, 512*c + 479 - x]
                dst = bass.AP(
                    tensor=G[:].tensor, offset=t0 * NFREE,
                    ap=[[gp, 128], [NFREE, 2], [1, NFREE]],
                )
                src = bass.AP(
                    tensor=gram[:].tensor, offset=NFREE - 1,
                    ap=[[pp, 128], [512, 2], [-1, NFREE]],
                )
                if t0 % 4 == 0:
                    nc.vector.tensor_copy(dst, src)
                else:
                    nc.scalar.copy(dst, src)
            # bounce: dense write; ONE 3-dim shifted read
            nc.sync.dma_start(scr[band, :, 0:GF], G[:])
            D2 = d_pool.tile([128, D2F], BF16, tag="D2")
            # init the xbar-padding tail, then the shifted read fills [0, D2RD)
            nc.gpsimd.memset(D2[:, D2F - 32 : D2F].bitcast(F32), 0.0)
            src = bass.AP(
                tensor=scr.tensor,
                offset=band * 128 * SCRF + 375,
                ap=[[32 * SCRF - 1, 4], [SCRF - 12, 32], [1, D2RD]],
            )
            dst = bass.AP(
                tensor=D2[:].tensor, offset=0,
                ap=[[D2F, 128], [1, D2RD]],
            )
            nc.scalar.dma_start(dst, src)
            return D2

        def output_band(band, D2, S):
            d2p = D2[:].ap[0][0]
            sp = S[:].ap[0][0]
            soff = (band % SBANDS) * SF
            for wq in range(NQ):
                use_pe = tr_mode == "pe" or (tr_mode == "mix" and wq % 2 == 0)
                if use_pe:
                    ptr = ps_tr.tile([128, 1024], BF16, tag="ptr")
                else:
                    ptr = t_pool.tile([128, 1024], BF16, tag="tsb")
                pp = ptr[:].ap[0][0]
                for wch in range(NWCH):
                    t = wq * NWCH + wch
                    if use_pe:
                        tin = bass.AP(
                            tensor=D2[:].tensor, offset=t * NFREE,
                            ap=[[d2p, 128], [1, RUN]],
                        )
                        nc.tensor.transpose(
                            bass.AP(
                                tensor=ptr[:].tensor, offset=128 * wch,
                                ap=[[pp, RUN], [1, 128]],
                            ),
                            tin, idt[:],
                        )
                    else:
                        tin = bass.AP(
                            tensor=D2[:].tensor, offset=t * NFREE,
                            ap=[[d2p, 128], [1, 128]],
                        )
                        tout = bass.AP(
                            tensor=ptr[:].tensor, offset=128 * wch,
                            ap=[[pp, 128], [1, 128]],
                        )
                        eng = nc.sync if wch % 2 == 0 else nc.scalar
                        eng.dma_start_transpose(tout, tin)
                # one batched evac for 8 transposes
                dst = bass.AP(
                    tensor=S[:].tensor,
                    offset=soff + (4 * wq) * W,
                    ap=[[sp, RUN], [32, NWCH], [W, 4], [1, 32]],
                )
                src = bass.AP(
                    tensor=ptr[:].tensor, offset=0,
                    ap=[[pp, RUN], [1, 1024]],
                )
                if wq % 2 == 0:
                    nc.scalar.copy(dst, src)
                else:
                    nc.vector.tensor_copy(dst, src)

        def output_rects(bwin, S, nb_in_win):
            # bands [bwin*SBANDS, +nb_in_win) -> out rows [h0, h0 + nb*BAND)
            h0 = bwin * SBANDS * BAND
            sp = S[:].ap[0][0]
            for b2, a_lo, na, ch0 in RECTS:
                src = bass.AP(
                    tensor=S[:].tensor,
                    offset=(12 * b2 + a_lo) * sp,
                    ap=[[sp, na], [1, nb_in_win * SF]],
                )
                dst = bass.AP(
                    tensor=out.tensor,
                    offset=ch0 * H_ * W + h0 * W,
                    ap=[[9 * H_ * W, na], [1, nb_in_win * SF]],
                )
                nc.gpsimd.dma_start(dst, src)

        for rep in range(reps):
            prev = None
            S_cur = None

            def emit_output(b, D2b):
                nonlocal S_cur
                if b % SBANDS == 0:
                    S_cur = s_pool.tile([128, SBANDS * SF], BF16, tag="S")
                output_band(b, D2b, S_cur)
                if b % SBANDS == SBANDS - 1 or b == nbands - 1:
                    output_rects(b // SBANDS, S_cur, b % SBANDS + 1)

            for band in range(nbands):
                D2 = compute_band(band)
                if prev is not None:
                    emit_output(band - 1, prev)
                prev = D2
            emit_output(nbands - 1, prev)
    nc.compile()
    return nc


def ref_one(x1, x2):
    """numpy reference for one batch element: x1, x2 [C, H, W] fp32."""
    C_, H_, W_ = x1.shape
    x2p = np.pad(x2, ((0, 0), (4, 4), (4, 4)))
    out = np.zeros((NCH, H_, W_), np.float32)
    for a in range(9):
        for b in range(9):
            ch = (40 - 9 * a - b) % NCH
            out[ch] = (x1 * x2p[:, a : a + H_, b : b + W_]).sum(0) / NCH
    return out


def prepare_in_maps(x1, x2):
    """Host-side prep: returns per-core input dicts for the bass kernel."""
    import ml_dtypes

    bf16 = ml_dtypes.bfloat16
    x1 = np.asarray(x1, dtype=np.float32)
    x2 = np.asarray(x2, dtype=np.float32)
    B, _, H_, W_ = x1.shape
    x1s = (x1 * (1.0 / NCH)).astype(bf16)
    # permute to [c, band, wq, wch, rho, q] and flatten to [C, H*W]
    x1r = (
        x1s.reshape(B, C, H_ // BAND, NQ, 4, NWCH, 32)
        .transpose(0, 1, 2, 3, 5, 4, 6)
        .reshape(B, C, H_ * W_)
    )
    x2p = np.zeros((B, C, H_ + 8, W_ + 8), dtype=bf16)
    x2p[:, :, 4:-4, 4:-4] = x2.astype(bf16)
    eye = np.eye(128, dtype=bf16)
    return [
        {"x1r": np.ascontiguousarray(x1r[i]),
         "x2p": np.ascontiguousarray(x2p[i]),
         "ident": eye}
        for i in range(B)
    ]


_NC_CACHE = {}


def _get_nc():
    if "nc" not in _NC_CACHE:
        _NC_CACHE["nc"] = build_cv_kernel(H_=H)
    return _NC_CACHE["nc"]


def kernel(x1, x2):
    """Full-input entry point: x1, x2 [8, 192, 128, 256] float32 ->
    [8, 81, 128, 256] float32. Data-parallel over batch: core i computes
    batch element i."""
    from concourse import bass_utils

    B = np.asarray(x1).shape[0]
    nc = _get_nc()
    in_maps = prepare_in_maps(x1, x2)
    res = bass_utils.run_bass_kernel_spmd(nc, in_maps, core_ids=list(range(B)))
    return np.stack([res.results[i]["out"] for i in range(B)], axis=0)
